# revision 30
# baseline (speedup 1.0000x reference)
"""Data-adaptive weight-ensembling MLP (per-sample expert-merged FFN) on 8 trn2 cores.

Math (per sample b):
  c[b,:,:]  = gate(x)[b].reshape(E, L)          (2-layer relu MLP gate)
  W1[b] = bW1 + sum_e c[b,e,0] tvW1[e];  b1[b] = bb1 + sum_e c[b,e,1] tvb1[e]
  W2[b] = bW2 + sum_e c[b,e,2] tvW2[e];  b2[b] = bb2 + sum_e c[b,e,3] tvb2[e]
  out[b] = relu(x[b] @ W1[b].T + b1[b]) @ W2[b].T + b2[b]

fp8 scheme (cfg="fp8", the default):
  c[b,e,l] = delta[b,e,l] + gb2[e,l]  with delta = h @ gW2.T (data part, no bias).
  The gb2-weighted expert sum is folded into the base weights ON HOST in f32:
      bW1' = bW1 + sum_e gb2[e,0] tvW1[e]   (shipped bf16)
  so the device-side task-vector stream only carries the small coefficients
  delta (std ~0.08). Task vectors ship as fp8 e4m3 scaled by S2=2^8; the
  per-sample stationary operand x1t[e][d,b] = x[b,d]*delta[b,e,0]*S1 (S1=2^6)
  is built on-chip in fp8. All matmuls accumulate into PSUM at scale
  SS=S1*S2=2^14 (base weights + biases pre-scaled by SS on host); the scale
  comes out for free in the relu/copy: relu(psum)*2^-14 == relu(psum*2^-14).
  fp8 x fp8 matmuls run in DoubleRow perf mode (K=256 per instruction, 2x
  moving-operand throughput): lhsT [128,2,16], rhs [128,2,512].

Sharding (8 cores): DFF=4096 split into 8 slices of 512. Core k computes
layer-1 output columns in its slice (full d-contraction locally -> exact
pre-activation -> local relu), then contracts layer 2 over the same f-slice.
Layer 2 runs per column half so each half's cross-core reduction overlaps
the other half's compute. Each half reduces hierarchically -- ReduceScatter
within 4-core groups (3 ring stages) then across group pairs (1 stage)
instead of a flat 8-core ring (7 stages); core k ends with output rows
4*(k%4)+2*(k//4)+{0,1}, reassembled on host. Collective-adjacent DMAs
ride the scalar (ACT) HWDGE ring so they never queue behind the task-vector
stream on the sync ring. Task-vector banks are sharded along DFF (16.8
MB/core fp8, 1MB DMA chunks, deep prefetch); merged base weights replicated
bf16, gate weights replicated fp8 (x2^9 scale folded into gb1 and the gate
relu's scale). Per-core HBM traffic ~19.7MB -> DMA-bound at ~358 GB/s/core
with PE (DoubleRow fp8), DVE, and collectives hidden behind the stream.
"""

import contextlib

import numpy as np

B, D, DFF, E, L = 16, 1024, 4096, 16, 4
NCORES = 8
OSL = DFF // NCORES          # 512: per-core DFF slice
KC1 = D // 128               # 8 k-chunks for the d contraction
KC2 = OSL // 128             # 4 k-chunks for the f contraction
EC = 2                       # experts per tv1 DMA chunk (1MB fp8 transfers)
NCH = E // EC                # 8 chunks
EC2 = 4                      # experts per tv2 DMA chunk (1MB per col-half)
NCH2 = E // EC2              # 4 chunks per column half
GS = 4                       # gate-shard group size (gW1 sharded 4-way)
GC = D // GS                 # 256: gate hidden cols computed per core
SG = 512.0                   # gW1 fp8 scale (2^9)
S1 = 64.0                    # x1t/x2t fp8 scale (2^6)
S2 = 256.0                   # tv fp8 scale (2^8)
SS = S1 * S2                 # psum scale (2^14)

_cache = {}


def _build_fp8(reps: int = 1, collective: bool = True,
               dma_only: bool = False, compute_only: bool = False,
               gate_shard: bool = False, gw1_fp8: bool = True):
    import concourse.bacc as bacc
    import concourse.bass as bass
    import concourse.tile as tile
    import concourse.mybir as mybir
    from concourse.masks import make_identity

    f32 = mybir.dt.float32
    bf16 = mybir.dt.bfloat16
    f8 = mybir.dt.float8e4
    DR = mybir.MatmulPerfMode.DoubleRow
    Relu = mybir.ActivationFunctionType.Relu
    nc = bacc.Bacc("TRN2", target_bir_lowering=False, debug=False,
                   num_devices=NCORES)

    # ---- I/O (per-core data supplied via in_maps) ----
    xT_h = nc.dram_tensor("xT", [128, KC1, B], bf16, kind="ExternalInput")
    GW = GC if gate_shard else D
    gdt = f8 if gw1_fp8 else bf16
    gsc = 1.0 / SG if gw1_fp8 else 1.0
    gw1_h = nc.dram_tensor("gw1", [128, KC1, GW], gdt,
                           kind="ExternalInput")
    gb1_h = nc.dram_tensor("gb1v", [1, GW], f32, kind="ExternalInput")
    if gate_shard:
        hg_in = nc.dram_tensor("hg_in", [B, GC], f32, kind="Internal")
        hg_out = nc.dram_tensor("hg_out", [GS, B, GC], f32, kind="Internal")
    gw2_h = nc.dram_tensor("gw2", [128, KC1, E * L], bf16, kind="ExternalInput")
    tv1_h = nc.dram_tensor("tv1", [NCH, 128, EC, KC1, OSL], f8,
                           kind="ExternalInput")
    bw1_h = nc.dram_tensor("bw1", [128, KC1, OSL], bf16, kind="ExternalInput")
    bb1_h = nc.dram_tensor("bb1v", [1, OSL], f32, kind="ExternalInput")
    tvb1_h = nc.dram_tensor("tvb1", [E, OSL], f32, kind="ExternalInput")
    tv2_h = nc.dram_tensor("tv2", [2, NCH2, 128, EC2, KC2, 512], f8,
                           kind="ExternalInput")
    bw2_h = nc.dram_tensor("bw2", [128, KC2, D], bf16, kind="ExternalInput")
    bb2_h = nc.dram_tensor("bb2v", [1, D], f32, kind="ExternalInput")
    tvb2_h = nc.dram_tensor("tvb2", [E, D], f32, kind="ExternalInput")
    out_h = nc.dram_tensor("out", [B // NCORES, D], f32,
                           kind="ExternalOutput")

    ar_in = [nc.dram_tensor(f"ar_in{n}", [B, 512], f32, kind="Internal")
             for n in range(2)]
    ar_mid = [nc.dram_tensor(f"ar_mid{n}", [B // 4, 512], f32,
                             kind="Internal") for n in range(2)]
    ar_out = [nc.dram_tensor(f"ar_out{n}", [B // NCORES, 512], f32,
                             kind="Internal") for n in range(2)]

    with tile.TileContext(nc) as tc, contextlib.ExitStack() as ctx:
        const = ctx.enter_context(tc.tile_pool(name="const", bufs=1))
        small = ctx.enter_context(tc.tile_pool(name="small", bufs=2))
        gwp = ctx.enter_context(tc.tile_pool(name="gwp", bufs=1))
        tvp1 = ctx.enter_context(tc.tile_pool(name="tvp1", bufs=6))
        tvp2 = ctx.enter_context(tc.tile_pool(name="tvp2", bufs=5))
        pacc = ctx.enter_context(tc.tile_pool(name="pacc", bufs=1,
                                              space="PSUM"))
        psml = ctx.enter_context(tc.tile_pool(name="psml", bufs=2,
                                              space="PSUM"))

        # constants (once)
        ones1 = const.tile([1, B], f32)
        nc.vector.memset(ones1[:], 1.0)
        ident16 = const.tile([B, B], f32)
        make_identity(nc, ident16[:])
        ones16_128 = const.tile([B, 128], f32)
        nc.vector.memset(ones16_128[:], 1.0)
        if compute_only:
            tvc1 = const.tile([128, EC, KC1, OSL], f8)
            nc.vector.memset(tvc1[:], 0.25)
            tvc2 = const.tile([128, EC2, KC2, 512], f8)
            nc.vector.memset(tvc2[:], 0.25)

        for _rep in range(reps):
            # small inputs
            xT = small.tile([128, KC1, B], bf16, name=f"xT_{_rep}", tag="xT")
            nc.sync.dma_start(out=xT[:], in_=xT_h.ap())
            gw1t = gwp.tile([128, KC1, GW], gdt, name=f"gw1t_{_rep}",
                            tag="gw1t")
            nc.sync.dma_start(out=gw1t[:], in_=gw1_h.ap())
            if _rep == 0 and not dma_only:
                # keep PE busy during the initial DMA so HAM un-throttles
                # before the real matmuls arrive (cold PE runs at 1.2 GHz)
                wps = pacc.tile([B, 128], f32, tag="psum2_1")
                for _ in range(32):
                    nc.tensor.matmul(wps[:], ones1[:], ones16_128[0:1, :],
                                     start=True, stop=True)
            gb1v = small.tile([1, GW], f32, name=f"gb1v_{_rep}", tag="gb1v")
            nc.sync.dma_start(out=gb1v[:], in_=gb1_h.ap())
            gw2t = small.tile([128, KC1, E * L], bf16, name=f"gw2t_{_rep}",
                              tag="gw2t")
            nc.sync.dma_start(out=gw2t[:], in_=gw2_h.ap())
            bw1s = small.tile([128, KC1, OSL], bf16, name=f"bw1s_{_rep}",
                              tag="bw1s")
            nc.sync.dma_start(out=bw1s[:], in_=bw1_h.ap())
            bb1v = small.tile([1, OSL], f32, name=f"bb1v_{_rep}", tag="bb1v")
            nc.sync.dma_start(out=bb1v[:], in_=bb1_h.ap())
            tvb1t = small.tile([E, OSL], f32, name=f"tvb1t_{_rep}", tag="tvb1t")
            nc.sync.dma_start(out=tvb1t[:], in_=tvb1_h.ap())
            bw2s = small.tile([128, KC2, D], bf16, name=f"bw2s_{_rep}",
                              tag="bw2s")
            nc.sync.dma_start(out=bw2s[:], in_=bw2_h.ap())
            bb2v = small.tile([1, D], f32, name=f"bb2v_{_rep}", tag="bb2v")
            nc.sync.dma_start(out=bb2v[:], in_=bb2_h.ap())
            tvb2t = small.tile([E, D], f32, name=f"tvb2t_{_rep}", tag="tvb2t")
            nc.sync.dma_start(out=tvb2t[:], in_=tvb2_h.ap())

            if dma_only:
                for c in range(NCH):
                    tvt = tvp1.tile([128, EC, KC1, OSL], f8, tag="tvt1")
                    nc.sync.dma_start(out=tvt[:], in_=tv1_h.ap()[c])
                for n in range(2):
                    for c in range(NCH2):
                        tvt2 = tvp2.tile([128, EC2, KC2, 512], f8,
                                         tag="tvt2")
                        nc.sync.dma_start(out=tvt2[:], in_=tv2_h.ap()[n][c])
                outp = small.tile([B, D], f32, name=f"outp_{_rep}",
                                  tag="outp")
                nc.vector.memset(outp[:], 0.0)
                if collective:
                    for n in range(2):
                        nc.scalar.dma_start(
                            out=ar_in[n].ap(),
                            in_=outp[:, n * 512:(n + 1) * 512])
                        nc.gpsimd.collective_compute(
                            "ReduceScatter", mybir.AluOpType.add,
                            replica_groups=[list(range(NCORES))],
                            ins=[ar_in[n].ap().opt()],
                            outs=[ar_out[n].ap().opt()],
                        )
                        nc.scalar.dma_start(
                            out=out_h.ap()[:, n * 512:(n + 1) * 512],
                            in_=ar_out[n].ap())
                else:
                    nc.scalar.dma_start(out=out_h.ap(),
                                        in_=outp[0:B // NCORES, :])
                continue

            # ---- gate layer 1 (gW1 sharded 4-way across core groups):
            # each core computes GC=256 cols of h = relu(x @ gW1.T + gb1),
            # then an AllGather over its group of 4 assembles the full h.
            # Column block g of h is computed by group-rank g, so the
            # gathered layout is rank-symmetric (same program on all cores).
            g_h = small.tile([B, D], f32, name=f"g_h_{_rep}", tag="g_h")
            if gate_shard:
                gps = pacc.tile([B, GC], f32, tag="gps")
                nc.tensor.matmul(gps[:], ones1[:], gb1v[:],
                                 start=True, stop=False)
                for kc in range(KC1):
                    nc.tensor.matmul(gps[:], xT[:, kc, :], gw1t[:, kc, :],
                                     start=False, stop=(kc == KC1 - 1))
                ghalf = small.tile([B, GC], f32, name=f"ghalf_{_rep}",
                                   tag="ghalf")
                nc.scalar.activation(ghalf[:], gps[:], Relu, scale=gsc)
                nc.scalar.dma_start(out=hg_in.ap(), in_=ghalf[:])
                nc.gpsimd.collective_compute(
                    "AllGather", mybir.AluOpType.bypass,
                    replica_groups=[[g * GS + j for j in range(GS)]
                                    for g in range(NCORES // GS)],
                    ins=[hg_in.ap().opt()],
                    outs=[hg_out.ap().opt()],
                )
                for g in range(GS):
                    nc.scalar.dma_start(out=g_h[:, g * GC:(g + 1) * GC],
                                        in_=hg_out.ap()[g])
            else:
                for n in range(2):
                    gps = pacc.tile([B, 512], f32, tag="gps")
                    nc.tensor.matmul(gps[:], ones1[:],
                                     gb1v[:, n * 512:(n + 1) * 512],
                                     start=True, stop=False)
                    for kc in range(KC1):
                        nc.tensor.matmul(
                            gps[:], xT[:, kc, :],
                            gw1t[:, kc, n * 512:(n + 1) * 512],
                            start=False, stop=(kc == KC1 - 1))
                    nc.scalar.activation(g_h[:, n * 512:(n + 1) * 512],
                                         gps[:], Relu, scale=gsc)

            # ---- transpose g_h -> ghT [128, (kc, b)] ----
            ghT = small.tile([128, KC1, B], bf16, name=f"ghT_{_rep}",
                             tag="ghT")
            for kc in range(KC1):
                pt = psml.tile([128, B], f32, tag="ps")
                nc.tensor.transpose(pt[:], g_h[:, kc * 128:(kc + 1) * 128],
                                    ident16[:])
                nc.vector.tensor_copy(ghT[:, kc, :], pt[:])

            # ---- gate layer 2 (NO bias): delta[b, e, l] = h @ gW2.T ----
            cps = psml.tile([B, E * L], f32, tag="ps")
            for kc in range(KC1):
                nc.tensor.matmul(cps[:], ghT[:, kc, :], gw2t[:, kc, :],
                                 start=(kc == 0), stop=(kc == KC1 - 1))
            cod = small.tile([B, E, L], f32, name=f"cod_{_rep}", tag="cod")
            nc.vector.tensor_copy(cod[:],
                                  cps[:].rearrange("b (e l) -> b e l", e=E))
            # scaled copy for the fp8 stationary path
            cods = small.tile([B, E, L], f32, name=f"cods_{_rep}", tag="cods")
            nc.vector.tensor_scalar_mul(cods[:], cod[:], S1)

            # ---- bias-coefficient matrices cT_l[e, b] = delta[b, e, l] ----
            cT = {}
            for l in (1, 3):
                cl = small.tile([B, E], f32, name=f"cl{l}_{_rep}",
                                tag=f"cl{l}")
                nc.vector.tensor_copy(cl[:], cod[:, :, l])
                ptc = psml.tile([B, E], f32, tag="ps")
                nc.tensor.transpose(ptc[:], cl[:], ident16[:])
                cTl = small.tile([E, B], f32, name=f"cT{l}_{_rep}",
                                 tag=f"cT{l}")
                nc.vector.tensor_copy(cTl[:], ptc[:])
                cT[l] = cTl

            # ---- broadcast tiles cbc[l][p, e, b] = delta[b, e, l]*S1 ----
            cbc = {}
            for l in (0, 2):
                diag_all = small.tile([B, E, B], f32,
                                      name=f"dga{l}_{_rep}", tag=f"dga{l}")
                for e in range(E):
                    nc.vector.tensor_scalar_mul(diag_all[:, e, :], ident16[:],
                                                cods[:, e, l:l + 1])
                pb = psml.tile([128, E * B], f32, tag="psb")
                nc.tensor.matmul(pb[:], ones16_128[:],
                                 diag_all[:].rearrange("b e c -> b (e c)"),
                                 start=True, stop=True)
                bc = small.tile([128, E, B], bf16, name=f"bc{l}_{_rep}",
                                tag=f"bc{l}")
                nc.vector.tensor_copy(bc[:],
                                      pb[:].rearrange("p (e c) -> p e c", e=E))
                cbc[l] = bc

            # ---- X1T[128, e, kc, b] = xT * delta0 * S1  (fp8) ----
            # split per DMA chunk so chunk 0's matmuls unblock early
            x1t = small.tile([128, E, KC1, B], f8, name=f"x1t_{_rep}",
                             tag="x1t")
            for c in range(NCH):
                e0 = c * EC
                nc.vector.tensor_mul(
                    x1t[:, e0:e0 + EC, :, :],
                    xT[:, None, :, :].broadcast_to([128, EC, KC1, B]),
                    cbc[0][:, e0:e0 + EC, None, :]
                    .broadcast_to([128, EC, KC1, B]))

            # ---- layer 1: psum1[b, o] at scale SS ----
            psum1 = pacc.tile([B, OSL], f32, tag="psum1")
            nc.tensor.matmul(psum1[:], ones1[:], bb1v[:], start=True,
                             stop=False)
            nc.tensor.matmul(psum1[:], cT[1][:], tvb1t[:], start=False,
                             stop=False)
            for kc in range(KC1):
                nc.tensor.matmul(psum1[:], xT[:, kc, :], bw1s[:, kc, :],
                                 start=False, stop=False)
            for c in range(NCH):
                if compute_only:
                    tvt = tvc1
                else:
                    tvt = tvp1.tile([128, EC, KC1, OSL], f8, tag="tvt1")
                    nc.sync.dma_start(out=tvt[:], in_=tv1_h.ap()[c])
                for je in range(EC):
                    e = c * EC + je
                    for kp in range(KC1 // 2):
                        nc.tensor.matmul(
                            psum1[:],
                            x1t[:, e, 2 * kp:2 * kp + 2, :],
                            tvt[:, je, 2 * kp:2 * kp + 2, :],
                            start=False,
                            stop=(c == NCH - 1 and je == EC - 1
                                  and kp == KC1 // 2 - 1),
                            perf_mode=DR)

            h1 = small.tile([B, OSL], f32, name=f"h1_{_rep}", tag="h1")
            nc.scalar.activation(h1[:], psum1[:], Relu, scale=1.0 / SS)

            # ---- transpose h1 -> h1T [128, (fc, b)] ----
            h1T = small.tile([128, KC2, B], bf16, name=f"h1T_{_rep}",
                             tag="h1T")
            for fc in range(KC2):
                pt2 = psml.tile([128, B], f32, tag="ps")
                nc.tensor.transpose(pt2[:], h1[:, fc * 128:(fc + 1) * 128],
                                    ident16[:])
                nc.vector.tensor_copy(h1T[:, fc, :], pt2[:])

            # ---- X2T[128, e, fc, b] = h1T * delta2 * S1  (fp8) ----
            x2t = small.tile([128, E, KC2, B], f8, name=f"x2t_{_rep}",
                             tag="x2t")
            for c in range(NCH2):
                e0 = c * EC2
                nc.vector.tensor_mul(
                    x2t[:, e0:e0 + EC2, :, :],
                    h1T[:, None, :, :].broadcast_to([128, EC2, KC2, B]),
                    cbc[2][:, e0:e0 + EC2, None, :]
                    .broadcast_to([128, EC2, KC2, B]))

            # ---- layer 2 by column half; overlap each half's collective
            # with the next half's compute ----
            outp = small.tile([B, D], f32, name=f"outp_{_rep}", tag="outp")
            for n in range(2):
                p = pacc.tile([B, 512], f32, tag=f"psum2_{n}")
                nc.tensor.matmul(p[:], ones1[:],
                                 bb2v[:, n * 512:(n + 1) * 512],
                                 start=True, stop=False)
                nc.tensor.matmul(p[:], cT[3][:],
                                 tvb2t[:, n * 512:(n + 1) * 512],
                                 start=False, stop=False)
                for fc in range(KC2):
                    nc.tensor.matmul(p[:], h1T[:, fc, :],
                                     bw2s[:, fc, n * 512:(n + 1) * 512],
                                     start=False, stop=False)
                for c in range(NCH2):
                    if compute_only:
                        tvt2 = tvc2
                    else:
                        tvt2 = tvp2.tile([128, EC2, KC2, 512], f8,
                                         tag="tvt2")
                        nc.sync.dma_start(out=tvt2[:], in_=tv2_h.ap()[n][c])
                    for je in range(EC2):
                        e = c * EC2 + je
                        for kp in range(KC2 // 2):
                            nc.tensor.matmul(
                                p[:],
                                x2t[:, e, 2 * kp:2 * kp + 2, :],
                                tvt2[:, je, 2 * kp:2 * kp + 2, :],
                                start=False,
                                stop=(c == NCH2 - 1 and je == EC2 - 1
                                      and kp == KC2 // 2 - 1),
                                perf_mode=DR)
                nc.vector.tensor_scalar_mul(outp[:, n * 512:(n + 1) * 512],
                                            p[:], 1.0 / SS)
                if collective:
                    nc.scalar.dma_start(out=ar_in[n].ap(),
                                        in_=outp[:, n * 512:(n + 1) * 512])
                    nc.gpsimd.collective_compute(
                        "ReduceScatter", mybir.AluOpType.add,
                        replica_groups=[[0, 1, 2, 3], [4, 5, 6, 7]],
                        ins=[ar_in[n].ap().opt()],
                        outs=[ar_mid[n].ap().opt()],
                    )
                    nc.gpsimd.collective_compute(
                        "ReduceScatter", mybir.AluOpType.add,
                        replica_groups=[[0, 4], [1, 5], [2, 6], [3, 7]],
                        ins=[ar_mid[n].ap().opt()],
                        outs=[ar_out[n].ap().opt()],
                    )
                    nc.scalar.dma_start(
                        out=out_h.ap()[:, n * 512:(n + 1) * 512],
                        in_=ar_out[n].ap())
            if not collective:
                nc.scalar.dma_start(out=out_h.ap(),
                                    in_=outp[0:B // NCORES, :])

    nc.compile()
    return nc


def _prep_fp8(x, gW1, gb1, gW2, gb2, bW1, bb1, bW2, bb2,
              tvW1, tvb1, tvW2, tvb2, gate_shard=False, gw1_fp8=True):
    """Build the 8 per-core in_maps (DMA-friendly layouts, fp8 tv)."""
    import ml_dtypes
    f = np.float32
    w = np.dtype(ml_dtypes.bfloat16)
    q8 = np.dtype(ml_dtypes.float8_e4m3)
    asf = lambda a: np.ascontiguousarray(a, dtype=f)
    asw = lambda a: np.ascontiguousarray(a.astype(f), dtype=w)
    asq = lambda a: np.ascontiguousarray(
        np.clip(a.astype(f) * f(S2), -240.0, 240.0), dtype=q8)

    x, gW1, gb1, gW2, gb2 = (np.asarray(t, f) for t in
                             (x, gW1, gb1, gW2, gb2))
    bW1, bb1, bW2, bb2 = (np.asarray(t, f) for t in (bW1, bb1, bW2, bb2))
    tvW1, tvb1, tvW2, tvb2 = (np.asarray(t, f) for t in
                              (tvW1, tvb1, tvW2, tvb2))

    gb2r = gb2.reshape(E, L)
    # fold the gb2-weighted expert sum into the base weights (f32 on host)
    bw1p = bW1 + (gb2r[:, 0] @ tvW1.reshape(E, -1)).reshape(DFF, D)
    bw2p = bW2 + (gb2r[:, 2] @ tvW2.reshape(E, -1)).reshape(D, DFF)
    bb1p = bb1 + gb2r[:, 1] @ tvb1
    bb2p = bb2 + gb2r[:, 3] @ tvb2

    asg = lambda a: np.ascontiguousarray(
        np.clip(a.astype(f) * f(SG), -240.0, 240.0), dtype=q8)
    xT = asw(x.T.reshape(KC1, 128, B).transpose(1, 0, 2))
    gw1full = (asg if gw1_fp8 else asw)(
        gW1.T.reshape(KC1, 128, D).transpose(1, 0, 2))
    gbsc = SG if gw1_fp8 else 1.0
    gw2 = asw(gW2.T.reshape(KC1, 128, E * L).transpose(1, 0, 2))

    in_maps = []
    for k in range(NCORES):
        o0 = k * OSL
        if gate_shard:
            g = k % GS
            gw1 = np.ascontiguousarray(gw1full[:, :, g * GC:(g + 1) * GC])
            gb1v = asf(gb1[g * GC:(g + 1) * GC].reshape(1, GC) * gbsc)
        else:
            gw1 = gw1full
            gb1v = asf(gb1.reshape(1, D) * gbsc)
        # [E, OSL, D] -> [NCH, 128, EC, KC1, OSL]
        tv1 = asq(tvW1[:, o0:o0 + OSL, :].transpose(0, 2, 1)
                  .reshape(NCH, EC, KC1, 128, OSL).transpose(0, 3, 1, 2, 4))
        bw1 = asw(bw1p[o0:o0 + OSL, :].T.reshape(KC1, 128, OSL)
                  .transpose(1, 0, 2) * f(SS))
        tv2 = asq(np.stack([
            tvW2[:, n * 512:(n + 1) * 512, o0:o0 + OSL].transpose(0, 2, 1)
            .reshape(NCH2, EC2, KC2, 128, 512).transpose(0, 3, 1, 2, 4)
            for n in range(2)]))
        bw2 = asw(bw2p[:, o0:o0 + OSL].T.reshape(KC2, 128, D)
                  .transpose(1, 0, 2) * f(SS))
        zero = k != 0
        in_maps.append(dict(
            xT=xT, gw1=gw1, gb1v=gb1v, gw2=gw2,
            tv1=tv1, bw1=bw1,
            bb1v=asf(bb1p[o0:o0 + OSL].reshape(1, OSL) * SS),
            tvb1=asf(tvb1[:, o0:o0 + OSL] * SS),
            tv2=tv2, bw2=bw2,
            bb2v=np.zeros((1, D), f) if zero else asf(bb2p.reshape(1, D) * SS),
            tvb2=np.zeros((E, D), f) if zero else asf(tvb2 * SS),
        ))
    return in_maps


# ---------------------------------------------------------------------------
# legacy bf16/f32 path (fallback)
# ---------------------------------------------------------------------------

def _build_bf16(reps: int = 1, collective: bool = True, cfg: str = "bf16"):
    import concourse.bacc as bacc
    import concourse.bass as bass
    import concourse.tile as tile
    import concourse.mybir as mybir
    from concourse.masks import make_identity

    f32 = mybir.dt.float32
    if cfg == "bf16":
        wdt = mybir.dt.bfloat16
        mmcast = lambda ap: ap
    elif cfg == "f32r":
        wdt = f32
        mmcast = lambda ap: ap.bitcast(mybir.dt.float32r)
    else:
        wdt = f32
        mmcast = lambda ap: ap
    Relu = mybir.ActivationFunctionType.Relu
    nc = bacc.Bacc("TRN2", target_bir_lowering=False, debug=False,
                   num_devices=NCORES)

    xT_h = nc.dram_tensor("xT", [128, KC1, B], wdt, kind="ExternalInput")
    gw1_h = nc.dram_tensor("gw1", [128, KC1, D], wdt, kind="ExternalInput")
    gb1_h = nc.dram_tensor("gb1v", [1, D], f32, kind="ExternalInput")
    gw2_h = nc.dram_tensor("gw2", [128, KC1, E * L], wdt, kind="ExternalInput")
    gb2_h = nc.dram_tensor("gb2v", [1, E * L], f32, kind="ExternalInput")
    tv1_h = nc.dram_tensor("tv1", [E, 128, KC1, OSL], wdt, kind="ExternalInput")
    bw1_h = nc.dram_tensor("bw1", [128, KC1, OSL], wdt, kind="ExternalInput")
    bb1_h = nc.dram_tensor("bb1v", [1, OSL], f32, kind="ExternalInput")
    tvb1_h = nc.dram_tensor("tvb1", [E, OSL], f32, kind="ExternalInput")
    tv2_h = nc.dram_tensor("tv2", [E, 128, KC2, D], wdt, kind="ExternalInput")
    bw2_h = nc.dram_tensor("bw2", [128, KC2, D], wdt, kind="ExternalInput")
    bb2_h = nc.dram_tensor("bb2v", [1, D], f32, kind="ExternalInput")
    tvb2_h = nc.dram_tensor("tvb2", [E, D], f32, kind="ExternalInput")
    out_h = nc.dram_tensor("out", [B, D], f32, kind="ExternalOutput")

    ar_in = nc.dram_tensor("ar_in", [B, D], f32, kind="Internal")
    ar_out = nc.dram_tensor("ar_out", [B, D], f32, kind="Internal",
                            addr_space="Shared")

    with tile.TileContext(nc) as tc, contextlib.ExitStack() as ctx:
        const = ctx.enter_context(tc.tile_pool(name="const", bufs=1))
        small = ctx.enter_context(tc.tile_pool(name="small", bufs=1))
        gwp = ctx.enter_context(tc.tile_pool(name="gwp", bufs=1))
        tvp1 = ctx.enter_context(tc.tile_pool(name="tvp1", bufs=3))
        tvp2 = ctx.enter_context(tc.tile_pool(name="tvp2", bufs=3))
        pacc = ctx.enter_context(tc.tile_pool(name="pacc", bufs=1,
                                              space="PSUM"))
        psml = ctx.enter_context(tc.tile_pool(name="psml", bufs=2,
                                              space="PSUM"))

        ones1 = const.tile([1, B], f32)
        nc.vector.memset(ones1[:], 1.0)
        ident16 = const.tile([B, B], f32)
        make_identity(nc, ident16[:])
        ones16_128 = const.tile([B, 128], f32)
        nc.vector.memset(ones16_128[:], 1.0)

        for _rep in range(reps):
            xT = small.tile([128, KC1, B], wdt, name=f"xT_{_rep}", tag="xT")
            nc.sync.dma_start(out=xT[:], in_=xT_h.ap())
            gb1v = small.tile([1, D], f32, name=f"gb1v_{_rep}", tag="gb1v")
            nc.sync.dma_start(out=gb1v[:], in_=gb1_h.ap())
            gb2v = small.tile([1, E * L], f32, name=f"gb2v_{_rep}", tag="gb2v")
            nc.sync.dma_start(out=gb2v[:], in_=gb2_h.ap())
            bb1v = small.tile([1, OSL], f32, name=f"bb1v_{_rep}", tag="bb1v")
            nc.sync.dma_start(out=bb1v[:], in_=bb1_h.ap())
            tvb1t = small.tile([E, OSL], f32, name=f"tvb1t_{_rep}", tag="tvb1t")
            nc.sync.dma_start(out=tvb1t[:], in_=tvb1_h.ap())
            bb2v = small.tile([1, D], f32, name=f"bb2v_{_rep}", tag="bb2v")
            nc.sync.dma_start(out=bb2v[:], in_=bb2_h.ap())
            tvb2t = small.tile([E, D], f32, name=f"tvb2t_{_rep}", tag="tvb2t")
            nc.sync.dma_start(out=tvb2t[:], in_=tvb2_h.ap())
            gw2t = small.tile([128, KC1, E * L], wdt, name=f"gw2t_{_rep}",
                              tag="gw2t")
            nc.sync.dma_start(out=gw2t[:], in_=gw2_h.ap())
            gw1t = gwp.tile([128, KC1, D], wdt, name=f"gw1t_{_rep}",
                            tag="gw1t")
            nc.sync.dma_start(out=gw1t[:], in_=gw1_h.ap())

            g_h = small.tile([B, D], f32, name=f"g_h_{_rep}", tag="g_h")
            for n in range(2):
                gps = pacc.tile([B, 512], f32, tag="gps")
                nc.tensor.matmul(gps[:], ones1[:],
                                 gb1v[:, n * 512:(n + 1) * 512],
                                 start=True, stop=False)
                for kc in range(KC1):
                    nc.tensor.matmul(gps[:], mmcast(xT[:, kc, :]),
                                     mmcast(gw1t[:, kc, n * 512:(n + 1) * 512]),
                                     start=False, stop=(kc == KC1 - 1))
                nc.scalar.activation(g_h[:, n * 512:(n + 1) * 512], gps[:],
                                     Relu)

            ghT = small.tile([128, KC1, B], wdt, name=f"ghT_{_rep}", tag="ghT")
            for kc in range(KC1):
                pt = psml.tile([128, B], f32, tag="ps")
                nc.tensor.transpose(pt[:], g_h[:, kc * 128:(kc + 1) * 128],
                                    ident16[:])
                nc.vector.tensor_copy(ghT[:, kc, :], pt[:])

            cps = psml.tile([B, E * L], f32, tag="ps")
            nc.tensor.matmul(cps[:], ones1[:], gb2v[:], start=True, stop=False)
            for kc in range(KC1):
                nc.tensor.matmul(cps[:], mmcast(ghT[:, kc, :]),
                                 mmcast(gw2t[:, kc, :]),
                                 start=False, stop=(kc == KC1 - 1))
            cod = small.tile([B, E, L], f32, name=f"cod_{_rep}", tag="cod")
            nc.vector.tensor_copy(cod[:],
                                  cps[:].rearrange("b (e l) -> b e l", e=E))

            cT = {}
            for l in (1, 3):
                cl = small.tile([B, E], f32, name=f"cl{l}_{_rep}",
                                tag=f"cl{l}")
                nc.vector.tensor_copy(cl[:], cod[:, :, l])
                ptc = psml.tile([B, E], f32, tag="ps")
                nc.tensor.transpose(ptc[:], cl[:], ident16[:])
                cTl = small.tile([E, B], f32, name=f"cT{l}_{_rep}",
                                 tag=f"cT{l}")
                nc.vector.tensor_copy(cTl[:], ptc[:])
                cT[l] = cTl

            cbc = {0: [], 2: []}
            for l in (0, 2):
                for e in range(E):
                    diag = small.tile([B, B], f32, name=f"dg{l}_{e}_{_rep}",
                                      tag="diag")
                    nc.vector.tensor_scalar_mul(diag[:], ident16[:],
                                                cod[:, e, l:l + 1])
                    pb = psml.tile([128, B], f32, tag="ps")
                    nc.tensor.matmul(pb[:], ones16_128[:], diag[:],
                                     start=True, stop=True)
                    bc = small.tile([128, B], wdt, name=f"bc{l}_{e}_{_rep}",
                                    tag=f"bc{l}_{e}")
                    nc.vector.tensor_copy(bc[:], pb[:])
                    cbc[l].append(bc)

            x1t = []
            for e in range(E):
                t = small.tile([128, KC1, B], wdt, name=f"x1t{e}_{_rep}",
                               tag=f"x1t{e}")
                nc.vector.tensor_mul(
                    t[:], xT[:],
                    cbc[0][e][:, None, :].broadcast_to([128, KC1, B]))
                x1t.append(t)

            psum1 = pacc.tile([B, OSL], f32, tag="psum1")
            nc.tensor.matmul(psum1[:], ones1[:], bb1v[:], start=True,
                             stop=False)
            nc.tensor.matmul(psum1[:], cT[1][:], tvb1t[:], start=False,
                             stop=False)
            for e in range(E + 1):
                tvt = tvp1.tile([128, KC1, OSL], wdt, tag="tvt1")
                nc.sync.dma_start(out=tvt[:],
                                  in_=bw1_h.ap() if e == E else tv1_h.ap()[e])
                lhs = xT if e == E else x1t[e]
                for kc in range(KC1):
                    nc.tensor.matmul(psum1[:], mmcast(lhs[:, kc, :]),
                                     mmcast(tvt[:, kc, :]),
                                     start=False,
                                     stop=(e == E and kc == KC1 - 1))

            h1 = small.tile([B, OSL], f32, name=f"h1_{_rep}", tag="h1")
            nc.scalar.activation(h1[:], psum1[:], Relu)

            h1T = small.tile([128, KC2, B], wdt, name=f"h1T_{_rep}", tag="h1T")
            for fc in range(KC2):
                pt2 = psml.tile([128, B], f32, tag="ps")
                nc.tensor.transpose(pt2[:], h1[:, fc * 128:(fc + 1) * 128],
                                    ident16[:])
                nc.vector.tensor_copy(h1T[:, fc, :], pt2[:])

            x2t = []
            for e in range(E):
                t = small.tile([128, KC2, B], wdt, name=f"x2t{e}_{_rep}",
                               tag=f"x2t{e}")
                nc.vector.tensor_mul(
                    t[:], h1T[:],
                    cbc[2][e][:, None, :].broadcast_to([128, KC2, B]))
                x2t.append(t)

            psum2 = []
            for n in range(2):
                p = pacc.tile([B, 512], f32, tag=f"psum2_{n}")
                nc.tensor.matmul(p[:], ones1[:],
                                 bb2v[:, n * 512:(n + 1) * 512],
                                 start=True, stop=False)
                nc.tensor.matmul(p[:], cT[3][:],
                                 tvb2t[:, n * 512:(n + 1) * 512],
                                 start=False, stop=False)
                psum2.append(p)
            for e in range(E + 1):
                tvt2 = tvp2.tile([128, KC2, D], wdt, tag="tvt2")
                nc.sync.dma_start(out=tvt2[:],
                                  in_=bw2_h.ap() if e == E else tv2_h.ap()[e])
                lhs = h1T if e == E else x2t[e]
                for fc in range(KC2):
                    for n in range(2):
                        nc.tensor.matmul(psum2[n][:], mmcast(lhs[:, fc, :]),
                                         mmcast(tvt2[:, fc, n * 512:(n + 1) * 512]),
                                         start=False,
                                         stop=(e == E and fc == KC2 - 1))

            outp = small.tile([B, D], f32, name=f"outp_{_rep}", tag="outp")
            for n in range(2):
                nc.vector.tensor_copy(outp[:, n * 512:(n + 1) * 512],
                                      psum2[n][:])

            if collective:
                nc.sync.dma_start(out=ar_in.ap(), in_=outp[:])
                nc.gpsimd.collective_compute(
                    "AllReduce", mybir.AluOpType.add,
                    replica_groups=[list(range(NCORES))],
                    ins=[ar_in.ap().opt()],
                    outs=[ar_out.ap().opt()],
                )
                nc.sync.dma_start(out=out_h.ap(), in_=ar_out.ap())
            else:
                nc.sync.dma_start(out=out_h.ap(), in_=outp[:])

    nc.compile()
    return nc


def _prep_bf16(x, gW1, gb1, gW2, gb2, bW1, bb1, bW2, bb2,
               tvW1, tvb1, tvW2, tvb2, cfg="bf16"):
    f = np.float32
    if cfg == "bf16":
        import ml_dtypes
        w = np.dtype(ml_dtypes.bfloat16)
    else:
        w = f
    asf = lambda a: np.ascontiguousarray(a, dtype=f)
    asw = lambda a: np.ascontiguousarray(np.asarray(a).astype(f), dtype=w)

    x, gW1, gb1, gW2, gb2 = (np.asarray(t, f) for t in
                             (x, gW1, gb1, gW2, gb2))
    bW1, bb1, bW2, bb2 = (np.asarray(t, f) for t in (bW1, bb1, bW2, bb2))
    tvW1, tvb1, tvW2, tvb2 = (np.asarray(t, f) for t in
                              (tvW1, tvb1, tvW2, tvb2))

    xT = asw(x.T.reshape(KC1, 128, B).transpose(1, 0, 2))
    gw1 = asw(gW1.T.reshape(KC1, 128, D).transpose(1, 0, 2))
    gw2 = asw(gW2.T.reshape(KC1, 128, E * L).transpose(1, 0, 2))
    gb1v = asf(gb1.reshape(1, D))
    gb2v = asf(gb2.reshape(1, E * L))

    in_maps = []
    for k in range(NCORES):
        o0 = k * OSL
        tv1 = asw(tvW1[:, o0:o0 + OSL, :].transpose(0, 2, 1)
                  .reshape(E, KC1, 128, OSL).transpose(0, 2, 1, 3))
        bw1 = asw(bW1[o0:o0 + OSL, :].T.reshape(KC1, 128, OSL)
                  .transpose(1, 0, 2))
        tv2 = asw(tvW2[:, :, o0:o0 + OSL].transpose(0, 2, 1)
                  .reshape(E, KC2, 128, D).transpose(0, 2, 1, 3))
        bw2 = asw(bW2[:, o0:o0 + OSL].T.reshape(KC2, 128, D)
                  .transpose(1, 0, 2))
        zero = k != 0
        in_maps.append(dict(
            xT=xT, gw1=gw1, gb1v=gb1v, gw2=gw2, gb2v=gb2v,
            tv1=tv1, bw1=bw1,
            bb1v=asf(bb1[o0:o0 + OSL].reshape(1, OSL)),
            tvb1=asf(tvb1[:, o0:o0 + OSL]),
            tv2=tv2, bw2=bw2,
            bb2v=np.zeros((1, D), f) if zero else asf(bb2.reshape(1, D)),
            tvb2=np.zeros((E, D), f) if zero else asf(tvb2),
        ))
    return in_maps


def _build(reps: int = 1, collective: bool = True, cfg: str = "fp8"):
    if cfg == "fp8":
        return _build_fp8(reps=reps, collective=collective)
    return _build_bf16(reps=reps, collective=collective, cfg=cfg)


def _prep_inputs(cfg="fp8", gate_shard=False, gw1_fp8=True, **inputs):
    if cfg == "fp8":
        return _prep_fp8(**inputs, gate_shard=gate_shard, gw1_fp8=gw1_fp8)
    return _prep_bf16(**inputs, cfg=cfg)


CFG = "fp8"


def kernel(**inputs):
    from concourse.bass_utils import run_bass_kernel_spmd

    key = ("nc", CFG)
    if key not in _cache:
        _cache[key] = _build(cfg=CFG)
    nc = _cache[key]

    in_maps = _prep_inputs(**{k: np.asarray(v) for k, v in inputs.items()},
                           cfg=CFG)
    res = run_bass_kernel_spmd(nc, in_maps, core_ids=list(range(NCORES)))
    if CFG == "fp8":
        return _assemble([res.results[k]["out"] for k in range(NCORES)])
    return res.results[0]["out"]


def _assemble(outs):
    """Hierarchical reduce-scatter leaves rows 4*(k%4)+2*(k//4) +{0,1}
    of the output on core k."""
    full = np.empty((B, D), np.float32)
    for k in range(NCORES):
        r0 = 4 * (k % 4) + 2 * (k // 4)
        full[r0:r0 + 2] = outs[k]
    return full
